# revision 1
# baseline (speedup 1.0000x reference)
"""MultiHeadAttention Trainium2 kernel (8 NeuronCores, SPMD, no collectives).

Reference model: B=4, S=2048, D=1024, H=16, Dh=64.
  q/k/v = split_heads(x @ W.T + b); scores = q k^T / sqrt(Dh); mask==0 -> -1e9;
  softmax; out = (attn v) @ fc_w.T + fc_b.

Sharding: core c handles batch b=c//2 and query rows [1024*(c%2), +1024).
K/V projections are recomputed per query-half (25% extra PE flops) which
avoids any collective: each core writes a disjoint [1024, 1024] output slice.

Layout strategy (per core):
  - Activations arrive host-transposed: x^T [d_in, t] so the PE (which
    contracts over partitions) can run every matmul without on-device
    transposes.
  - Q^T, K^T [d_out, t] produced directly by projection matmuls.
  - V produced in natural [t, d_out] layout (serves as lhsT of the PV matmul),
    stored with a ones column per head (66-wide groups) so the PV matmul also
    emits the softmax row-sums (row 64 of each PV psum).
  - scores^T [tk, tq] in PSUM; exp runs on ScalarE straight out of PSUM with
    the 1/8 scale folded into the activation's free affine; mask (0/1 bf16)
    applied multiplicatively by VectorE after exp (mathematically identical
    to the -1e9 additive mask since softmax is shift/scale invariant here
    and scores are tiny: |s/8| < ~3, so skipping the row-max is safe).
  - normalization: recip(rowsum) broadcast across partitions with a K=1 ones
    matmul; V-bias folded in as pv += bv (x) rowsum (rank-1 matmul) so the
    final normalize is a single tensor_tensor multiply.
  - fc bias folded in as a K=1 ones (x) fc_b matmul accumulation.
"""

import os

import numpy as np
import ml_dtypes

BF16 = ml_dtypes.bfloat16

D = 1024
S = 2048
B = 4
H = 16
DH = 64
TQ = 1024  # query rows per core
P = 128
N_CORES = 8

_CACHED = {}


def _build():
    import concourse.bass as bass
    import concourse.mybir as mybir
    import concourse.tile as tile
    from concourse import bacc

    BF = mybir.dt.bfloat16
    F32 = mybir.dt.float32
    F32R = mybir.dt.float32r
    AF = mybir.ActivationFunctionType

    nc = bacc.Bacc("TRN2", target_bir_lowering=False, debug=False)

    xqT = nc.dram_tensor("xqT", [D, TQ], BF, kind="ExternalInput").ap()
    xkT = nc.dram_tensor("xkT", [D, S], BF, kind="ExternalInput").ap()
    xvT = nc.dram_tensor("xvT", [D, S], BF, kind="ExternalInput").ap()
    wqT = nc.dram_tensor("wqT", [D, D], BF, kind="ExternalInput").ap()
    wkT = nc.dram_tensor("wkT", [D, D], BF, kind="ExternalInput").ap()
    wvT = nc.dram_tensor("wvT", [D, D], BF, kind="ExternalInput").ap()
    fcT = nc.dram_tensor("fcT", [D, D], BF, kind="ExternalInput").ap()
    maskT = nc.dram_tensor("maskT", [S, TQ], BF, kind="ExternalInput").ap()
    bq_d = nc.dram_tensor("bq", [P, 8], F32, kind="ExternalInput").ap()
    bk_d = nc.dram_tensor("bk", [P, 8], F32, kind="ExternalInput").ap()
    bv_d = nc.dram_tensor("bv", [1, D], F32, kind="ExternalInput").ap()
    fcb_d = nc.dram_tensor("fcb", [1, D], BF, kind="ExternalInput").ap()
    out = nc.dram_tensor("out", [TQ, D], F32, kind="ExternalOutput").ap()

    VGW = 66  # per-head group width in V storage: 64 V cols + ones col + pad

    with tile.TileContext(nc) as tc:
        with tc.tile_pool(name="const", bufs=1) as const:
            # Persistent SBUF tensors
            QT = const.tile([P, 8, TQ], BF)       # Q^T  (d_out, tq)
            KT = const.tile([P, 8, S], BF)        # K^T  (d_out, tk)
            VG = const.tile([P, 16, H * VGW], BF)  # V (+ones), tk-tiled
            AOT = const.tile([P, 8, TQ], BF)      # attn-out^T (d_out, tq)
            FCT = const.tile([P, 8, D], BF)       # fc_w^T
            bq_s = const.tile([P, 8], F32)
            bk_s = const.tile([P, 8], F32)
            bv_s = const.tile([1, D], F32R)
            bv_tmp = const.tile([1, D], F32)
            ones_tmp = const.tile([1, 64], F32)
            fcb_s = const.tile([1, D], BF)
            ones_bf = const.tile([1, P], BF)
            ones_f32 = const.tile([1, 64], F32R)

            nc.sync.dma_start(bq_s[:], bq_d)
            nc.sync.dma_start(bk_s[:], bk_d)
            nc.sync.dma_start(bv_tmp[:], bv_d)
            nc.vector.tensor_copy(bv_s[:], bv_tmp[:])
            nc.sync.dma_start(fcb_s[:], fcb_d)
            nc.vector.memset(ones_bf[:], 1.0)
            nc.vector.memset(ones_tmp[:], 1.0)
            nc.vector.tensor_copy(ones_f32[:], ones_tmp[:])
            # ones columns for V (memset everything, V values overwrite)
            for tt in range(16):
                nc.vector.memset(VG[:, tt], 1.0)
            nc.sync.dma_start(FCT[:], fcT.rearrange("(j p) n -> p j n", p=P))

            # ---------------- projections ----------------
            with (
                tc.tile_pool(name="xin", bufs=8) as xin,
                tc.tile_pool(name="wpool", bufs=1) as wpool,
                tc.tile_pool(name="ppsum", bufs=2, space="PSUM") as ppsum,
            ):
                # V projection: V[t, do] = sum_di xvT[di,t] * wvT[di,do]
                wv_s = wpool.tile([P, 8, D], BF, tag="w")
                nc.sync.dma_start(wv_s[:], wvT.rearrange("(j p) n -> p j n", p=P))
                xv = []
                for di in range(8):
                    t_ = xin.tile([P, S], BF, tag="xt")
                    nc.sync.dma_start(t_[:], xvT[di * P:(di + 1) * P, :])
                    xv.append(t_)
                for tt in range(16):
                    for n in range(2):
                        ps = ppsum.tile([P, 512], F32, tag="pp")
                        for di in range(8):
                            nc.tensor.matmul(
                                ps[:],
                                lhsT=xv[di][:, tt * P:(tt + 1) * P],
                                rhs=wv_s[:, di, n * 512:(n + 1) * 512],
                                start=(di == 0),
                                stop=(di == 7),
                            )
                        # scatter 8 heads' 64-wide chunks into 66-wide groups
                        nc.vector.tensor_copy(
                            VG[:, tt].rearrange("p (h c) -> p h c", c=VGW)[
                                :, n * 8:(n + 1) * 8, :64
                            ],
                            ps.rearrange("p (h c) -> p h c", c=64),
                        )

                # K projection: K^T[do, tk] = sum_di wkT[di,do] * xkT[di,tk]
                wk_s = wpool.tile([P, 8, D], BF, tag="w")
                nc.sync.dma_start(wk_s[:], wkT.rearrange("(j p) n -> p j n", p=P))
                xk = []
                for di in range(8):
                    t_ = xin.tile([P, S], BF, tag="xt")
                    nc.sync.dma_start(t_[:], xkT[di * P:(di + 1) * P, :])
                    xk.append(t_)
                for j in range(8):
                    for n in range(4):
                        ps = ppsum.tile([P, 512], F32, tag="pp")
                        for di in range(8):
                            nc.tensor.matmul(
                                ps[:],
                                lhsT=wk_s[:, di, j * P:(j + 1) * P],
                                rhs=xk[di][:, n * 512:(n + 1) * 512],
                                start=(di == 0),
                                stop=(di == 7),
                            )
                        nc.vector.tensor_scalar_add(
                            KT[:, j, n * 512:(n + 1) * 512], ps[:], bk_s[:, j:j + 1]
                        )

                # Q projection
                wq_s = wpool.tile([P, 8, D], BF, tag="w")
                nc.sync.dma_start(wq_s[:], wqT.rearrange("(j p) n -> p j n", p=P))
                xq = []
                for di in range(8):
                    t_ = xin.tile([P, S], BF, tag="xt")
                    nc.sync.dma_start(t_[:, :TQ], xqT[di * P:(di + 1) * P, :])
                    xq.append(t_)
                for j in range(8):
                    for n in range(2):
                        ps = ppsum.tile([P, 512], F32, tag="pp")
                        for di in range(8):
                            nc.tensor.matmul(
                                ps[:],
                                lhsT=wq_s[:, di, j * P:(j + 1) * P],
                                rhs=xq[di][:, n * 512:(n + 1) * 512],
                                start=(di == 0),
                                stop=(di == 7),
                            )
                        nc.vector.tensor_scalar_add(
                            QT[:, j, n * 512:(n + 1) * 512], ps[:], bq_s[:, j:j + 1]
                        )

            # ---------------- attention ----------------
            with (
                tc.tile_pool(name="spsum", bufs=2, space="PSUM") as spsum,
                tc.tile_pool(name="vpsum", bufs=2, space="PSUM") as vpsum,
                tc.tile_pool(name="bpsum", bufs=2, space="PSUM") as bpsum,
                tc.tile_pool(name="ppool", bufs=3) as ppool,
                tc.tile_pool(name="npool", bufs=2) as npool,
                tc.tile_pool(name="mpool", bufs=1) as mpool,
            ):
                MSK = mpool.tile([P, 16, TQ], BF)     # mask^T, tk-tiled
                nc.sync.dma_start(
                    MSK[:], maskT.rearrange("(t p) q -> p t q", p=P)
                )
                for h in range(H):
                    j, bp = h // 2, 64 * (h % 2)
                    pv = []
                    for _pvi in range(2):
                        pv_t = vpsum.tile([65, 512], F32, tag="pv", name=f"pv{_pvi}")
                        pv.append(pv_t)
                    for tk in range(16):
                        sc = spsum.tile([P, TQ], F32, tag="sc")
                        for n in range(2):
                            nc.tensor.matmul(
                                sc[:, n * 512:(n + 1) * 512],
                                lhsT=KT[bp:bp + 64, j, tk * P:(tk + 1) * P],
                                rhs=QT[bp:bp + 64, j, n * 512:(n + 1) * 512],
                                start=True,
                                stop=True,
                            )
                        pt = ppool.tile([P, TQ], BF, tag="pt")
                        nc.scalar.activation(pt[:], sc[:], AF.Exp, scale=0.125)
                        nc.vector.tensor_mul(pt[:], pt[:], MSK[:, tk])
                        for n in range(2):
                            nc.tensor.matmul(
                                pv[n][:],
                                lhsT=VG[:, tk, h * VGW:h * VGW + 65],
                                rhs=pt[:, n * 512:(n + 1) * 512],
                                start=(tk == 0),
                                stop=False,
                            )
                    # rowsums -> SBUF, reciprocal
                    rs = npool.tile([1, TQ], F32R, tag="rs")
                    for n in range(2):
                        nc.vector.tensor_copy(
                            rs[:, n * 512:(n + 1) * 512], pv[n][64:65, :]
                        )
                    rc = npool.tile([1, TQ], F32, tag="rc")
                    nc.vector.reciprocal_approx_fast(rc[:], rs[:].bitcast(F32))
                    rcr = npool.tile([1, TQ], F32R, tag="rcr")
                    nc.vector.tensor_copy(rcr[:], rc[:])
                    for n in range(2):
                        # pv[:64] += bv_head (x) rowsum  (rank-1), so that
                        # (pv + bv*rs) * (1/rs) = pv/rs + bv
                        nc.tensor.matmul(
                            pv[n][:64, :],
                            lhsT=bv_s[:, h * 64:(h + 1) * 64],
                            rhs=rs[:, n * 512:(n + 1) * 512],
                            start=False,
                            stop=True,
                        )
                        # broadcast recip across 64 partitions via K=1 matmul
                        bc = bpsum.tile([64, 512], F32, tag="bc")
                        nc.tensor.matmul(
                            bc[:],
                            lhsT=ones_f32[:],
                            rhs=rcr[:, n * 512:(n + 1) * 512],
                            start=True,
                            stop=True,
                        )
                        bcs = npool.tile([64, 512], F32, tag="bcs")
                        nc.vector.tensor_copy(bcs[:], bc[:])
                        nc.vector.tensor_mul(
                            AOT[bp:bp + 64, j, n * 512:(n + 1) * 512],
                            pv[n][:64, :],
                            bcs[:],
                        )

            # ---------------- output projection ----------------
            with (
                tc.tile_pool(name="fpsum", bufs=2, space="PSUM") as fpsum,
                tc.tile_pool(name="opool", bufs=2) as opool,
            ):
                for tt in range(8):
                    ob = opool.tile([P, D], F32, tag="ob")
                    for n in range(2):
                        ps = fpsum.tile([P, 512], F32, tag="fp")
                        for j in range(8):
                            nc.tensor.matmul(
                                ps[:],
                                lhsT=AOT[:, j, tt * P:(tt + 1) * P],
                                rhs=FCT[:, j, n * 512:(n + 1) * 512],
                                start=(j == 0),
                                stop=False,
                            )
                        nc.tensor.matmul(
                            ps[:],
                            lhsT=ones_bf[:],
                            rhs=fcb_s[:, n * 512:(n + 1) * 512],
                            start=False,
                            stop=True,
                        )
                        nc.vector.tensor_copy(ob[:, n * 512:(n + 1) * 512], ps[:])
                    nc.sync.dma_start(out[tt * P:(tt + 1) * P, :], ob[:])

    nc.compile()
    return nc


def _get_nc():
    if "nc" not in _CACHED:
        _CACHED["nc"] = _build()
    return _CACHED["nc"]


def kernel(**inputs):
    from concourse import bass_utils

    query = np.asarray(inputs["query"], np.float32)
    key_in = np.asarray(inputs["key_in"], np.float32)
    value = np.asarray(inputs["value"], np.float32)
    mask = np.asarray(inputs["mask"])
    wq_w = np.asarray(inputs["wq_w"], np.float32)
    wq_b = np.asarray(inputs["wq_b"], np.float32)
    wk_w = np.asarray(inputs["wk_w"], np.float32)
    wk_b = np.asarray(inputs["wk_b"], np.float32)
    wv_w = np.asarray(inputs["wv_w"], np.float32)
    wv_b = np.asarray(inputs["wv_b"], np.float32)
    fc_w = np.asarray(inputs["fc_w"], np.float32)
    fc_b = np.asarray(inputs["fc_b"], np.float32)

    def c(a):
        return np.ascontiguousarray(a)

    shared = {
        "wqT": c(wq_w.T.astype(BF16)),
        "wkT": c(wk_w.T.astype(BF16)),
        "wvT": c(wv_w.T.astype(BF16)),
        "fcT": c(fc_w.T.astype(BF16)),
        "bq": c(wq_b.reshape(8, P).T.astype(np.float32)),
        "bk": c(wk_b.reshape(8, P).T.astype(np.float32)),
        "bv": c(wv_b.reshape(1, D).astype(np.float32)),
        "fcb": c(fc_b.reshape(1, D).astype(BF16)),
    }

    in_maps = []
    for core in range(N_CORES):
        b, q0 = core // 2, TQ * (core % 2)
        m = dict(shared)
        m["xqT"] = c(query[b].T[:, q0:q0 + TQ].astype(BF16))
        m["xkT"] = c(key_in[b].T.astype(BF16))
        m["xvT"] = c(value[b].T.astype(BF16))
        m["maskT"] = c(mask[b][q0:q0 + TQ, :].T.astype(BF16))
        in_maps.append(m)

    nc = _get_nc()
    trace = bool(int(os.environ.get("KERNEL_TRACE", "0")))
    res = bass_utils.run_bass_kernel_spmd(
        nc, in_maps, core_ids=list(range(N_CORES)), trace=trace,
        **({"trace_cores": [0]} if trace else {}),
    )
    _CACHED["last_results"] = res

    full = np.empty((B, S, D), np.float32)
    for core in range(N_CORES):
        b, q0 = core // 2, TQ * (core % 2)
        full[b, q0:q0 + TQ, :] = res.results[core]["out"]
    return full

